# revision 56
# baseline (speedup 1.0000x reference)
"""Multi-head causal attention (B=2, C=2048, E=1024, H=16) on 8 NeuronCores.

Sharding: tensor-parallel over (batch, head-group): core = b*4 + g handles
batch b and heads [4g, 4g+4). Each core computes Q^T/K^T/V projections for
its 4 heads, causal attention, and its partial output projection
ctx_slice @ Wo_slice -> [2048, 1024]. Host sums the 4 partials per batch
(the tensor-parallel all-reduce, done at unshard time) and adds bo.

Dataflow is fully transposed so no on-device transposes are needed:
  Q^T = Wq_s.T @ x^T          [256 f, 2048 t]   (f = head-local features)
  K^T = Wk_s.T @ x^T          [256 f, 2048 t]
  V   = x    @ Wv_s           [2048 t, 256 f]  (natural layout, + ones col)
  s^T = K^T_h.T @ Q^T_h       [128 k, 512 q] per (head, k-chunk, q-tile)
  P^T = exp(s^T / 32) * mask  (no max-subtraction: |scores/32| < ~2.5)
  ctx_aug^T = V_aug.T @ P^T   [65, 512], row 64 = softmax normalizer l
  ctx^T = ctx_aug^T[0:64] * (1/l)  (broadcast via gpsimd)
  out_partial = ctx^T.T @ Wo_s     [2048, 1024] fp32

All matmul operands are fp16 (PE upconverts to FP22, accumulates fp32 in
PSUM): end-to-end max rel err vs fp64 reference is ~4e-4.
"""
import numpy as np

import concourse.bass as bass
import concourse.tile as tile
from concourse import bacc, mybir
from concourse.bass_utils import run_bass_kernel_spmd

F16 = mybir.dt.float16
F32 = mybir.dt.float32

B, C, E, H = 2, 2048, 1024, 16
NH = 4              # heads per core
D = 64              # head dim
FS = NH * D         # 256 features per core
EC = E // 128       # 8 e-chunks
QT = 512            # q tile size
NQ = C // QT        # 4 q tiles
KC = C // 128       # 16 k chunks
TC = C // 128       # 16 token chunks
SCALE = 1.0 / np.sqrt(np.float32(E))  # note: module scales by sqrt(E)

_CACHED_NC = None


def build():
    nc = bacc.Bacc("TRN2", target_bir_lowering=False, debug=False, num_devices=8)
    # xT arrives token-block-major: [tb, c, 128, 512] so each (tb, c) piece
    # is one contiguous 128KB DMA and projections can pace to block arrival
    xT = nc.dram_tensor("xT", [NQ, EC, 128, QT], F16, kind="ExternalInput")
    # weights/masks arrive pre-laid-out for contiguous DMA:
    # wq/wk/wv: [128, EC, FS]; wo: [128, 2, E]; msk: [128, 4, QT]
    wq = nc.dram_tensor("wq", [128, EC, FS], F16, kind="ExternalInput")
    wk = nc.dram_tensor("wk", [128, EC, FS], F16, kind="ExternalInput")
    wv = nc.dram_tensor("wv", [128, EC, FS], F16, kind="ExternalInput")
    wo = nc.dram_tensor("wo", [128, 2, E], F16, kind="ExternalInput")
    msk = nc.dram_tensor("msk", [128, 128], F16, kind="ExternalInput")
    out = nc.dram_tensor("out", [E, C], F16, kind="ExternalOutput")  # out^T

    with tile.TileContext(nc) as tc:
        with tc.tile_pool(name="const", bufs=1) as cp, \
             tc.tile_pool(name="work", bufs=1) as wp, \
             tc.tile_pool(name="ps", bufs=1, space="PSUM") as ps:
            # ---- resident SBUF tensors ----
            xT_sb = cp.tile([128, EC, C], F16)
            wq_sb = cp.tile([128, EC, FS], F16)
            wk_sb = cp.tile([128, EC, FS], F16)
            wv_sb = cp.tile([128, EC, FS], F16)
            wo_sb = cp.tile([128, 2, E], F16)
            msk_sb = cp.tile([128, 128], F16)
            qt_sb = cp.tile([128, 2, C], F16)
            kt_sb = cp.tile([128, 2, C], F16)
            v_sb = cp.tile([128, TC, NH * (D + 1)], F16)   # +1: ones col per head
            ctxt_sb = cp.tile([128, 2, C], F16)

            # ---- input DMAs in first-consumption order.  DMA data is
            # striped across all engines, so arrival ~ emission order at
            # the aggregate rate: wk ~9.4us, tb0 ~12.2, wq ~13.6, ...
            nc.sync.dma_start(wk_sb[:], wk[:])
            for c in range(EC):
                nc.sync.dma_start(xT_sb[:, c, 0:QT], xT[0, c])
            nc.sync.dma_start(wq_sb[:], wq[:])
            nc.sync.dma_start(msk_sb[:], msk[:])
            nc.sync.dma_start(wv_sb[:], wv[:])
            for tb in range(1, NQ):
                for c in range(EC):
                    nc.sync.dma_start(
                        xT_sb[:, c, QT * tb:QT * (tb + 1)], xT[tb, c])
            nc.sync.dma_start(wo_sb[:], wo[:])
            nc.vector.memset(v_sb[:], 1.0)  # ones cols survive the V copy

            # ---- PE warm-up: ~7us of dummy matmuls during the input-DMA
            # window so HAM un-throttles before the real stream starts
            wu = wp.tile([128, 512], F16, tag="wu", bufs=1)
            nc.vector.memset(wu[:], 0.5)
            ones = wp.tile([1, 64], F16, tag="ones", bufs=1)
            nc.vector.memset(ones[:], 1.0)
            wups = ps.tile([128, 512], F32, tag="ctx", bufs=2, name="wups")
            for i in range(16):
                nc.tensor.matmul(wups[:], lhsT=wu[:, 0:128], rhs=wu[:],
                                 start=True, stop=True)

            # ---- projection building blocks (emitted interleaved below) ----
            def proj_kq(w_sb, o_sb, g2, j):
                pp = ps.tile([128, QT], F32, tag="pj", bufs=2,
                             name=f"pp_{o_sb.name}_{g2}_{j}")
                for c in range(EC):
                    nc.tensor.matmul(
                        pp[:],
                        lhsT=w_sb[:, c, 128 * g2:128 * (g2 + 1)],
                        rhs=xT_sb[:, c, QT * j:QT * (j + 1)],
                        start=(c == 0), stop=(c == EC - 1),
                    )
                nc.vector.tensor_copy(o_sb[:, g2, QT * j:QT * (j + 1)], pp[:])

            def proj_v(t):
                pp = ps.tile([128, FS], F32, tag="pj", bufs=2,
                             name=f"pp_v_{t}")
                for c in range(EC):
                    nc.tensor.matmul(
                        pp[:],
                        lhsT=xT_sb[:, c, 128 * t:128 * (t + 1)],
                        rhs=wv_sb[:, c, :],
                        start=(c == 0), stop=(c == EC - 1),
                    )
                nc.vector.tensor_copy(
                    v_sb[:, t, :].rearrange("p (h x) -> p h x", h=NH)[:, :, 0:D],
                    pp[:].rearrange("p (h d) -> p h d", h=NH),
                )

            # ---- attention: head pairs (0,1)/(2,3); both heads' score tiles
            # ---- share one [128, 2*QT] psum so exp+mask are single wide ops
            def emit_scores(heads, j, c):
                """s^T pair -> one exp -> (one mask) -> fp16 P^T [128, 2*QT].

                Diagonal chunks (c = 4j + r): queries q < 128r are fully
                masked, so all work is restricted to q in [128r, QT)."""
                q0 = 128 * (c - 4 * j) if c >= 4 * j else 0
                qw = QT - q0
                st = ps.tile([128, 2 * QT], F32, tag="big", bufs=2,
                             name=f"st_{heads[0]}_{j}_{c}")
                for i, h in enumerate(heads):
                    g2, po = h // 2, 64 * (h % 2)
                    nc.tensor.matmul(
                        st[:, QT * i + q0:QT * (i + 1)],
                        lhsT=kt_sb[po:po + 64, g2, 128 * c:128 * (c + 1)],
                        rhs=qt_sb[po:po + 64, g2, QT * j + q0:QT * (j + 1)],
                        start=True, stop=True,
                    )
                pt = wp.tile([128, 2 * QT], F16, tag="pt", bufs=6)
                st3 = st[:].rearrange("p (b q) -> p b q", b=2)[:, :, q0:QT]
                pt3 = pt[:].rearrange("p (b q) -> p b q", b=2)[:, :, q0:QT]
                nc.scalar.activation(
                    pt3, st3, mybir.ActivationFunctionType.Exp, scale=SCALE)
                if c >= 4 * j:  # mask the diagonal 128-wide block in place
                    # (the causal mask within that block is the same
                    # [128,128] lower-tri for every diagonal chunk)
                    ptd = pt[:].rearrange("p (b q) -> p b q", b=2)[
                        :, :, q0:q0 + 128]
                    nc.vector.tensor_mul(
                        ptd, ptd,
                        msk_sb[:].unsqueeze(1).broadcast_to([128, 2, 128]))
                return pt

            def attention(pair, j, injects=(), prev_norm=None, last=False):
                """injects: callables emitted one per k-chunk iteration --
                extra tensor work scheduled into exp-bound PE bubbles.
                prev_norm: the previous call's deferred normalize chain,
                emitted after this call's prologue so its bc matmuls don't
                head-of-line-block the prologue scores in the tensor queue.
                Returns this call's normalize closure (None when last=True,
                which emits it inline and shortens the chain)."""
                heads = (2 * pair, 2 * pair + 1)
                nk = 4 * (j + 1)   # causal: k chunks 0..nk-1
                ctx_ps = {h: ps.tile([128, QT], F32, tag="ctx", bufs=2,
                                     name=f"ctx_{pair}_{j}_{h}")
                          for h in heads}
                pts = {}
                depth = min(2, nk)
                for c in range(depth):          # software-pipeline prologue
                    pts[c] = emit_scores(heads, j, c)
                if prev_norm is not None:
                    prev_norm()
                inj = list(injects)
                for c in range(nk):
                    if c < len(inj):
                        inj[c]()
                    if c + depth < nk:
                        pts[c + depth] = emit_scores(heads, j, c + depth)
                    pt = pts.pop(c)
                    q0 = 128 * (c - 4 * j) if c >= 4 * j else 0
                    for i, h in enumerate(heads):
                        nc.tensor.matmul(
                            ctx_ps[h][0:D + 1, q0:QT],
                            lhsT=v_sb[:, c, (D + 1) * h:(D + 1) * (h + 1)],
                            rhs=pt[:, QT * i + q0:QT * (i + 1)],
                            start=(c == 0), stop=(c == nk - 1),
                        )
                for fn in inj[nk:]:
                    fn()

                def norm():
                    # normalize: ctx^T[d, q] * (1/l[q]).  Stage both heads'
                    # psum to SBUF first (frees the ctx slots fast); l is
                    # broadcast across partitions with a rank-1 PE matmul --
                    # lower latency than the gpsimd partition_broadcast.
                    if last:
                        stgs = {h: ctx_ps[h] for h in heads}
                    else:
                        stgs = {}
                        for h in heads:
                            stg = wp.tile([D + 1, QT], F32, tag="stg",
                                          bufs=4, name=f"stg_{pair}_{j}_{h}")
                            nc.vector.tensor_copy(stg[:],
                                                  ctx_ps[h][0:D + 1, :])
                            stgs[h] = stg
                    lr = wp.tile([1, 2 * QT], F16, tag="lr", bufs=4)
                    rc = wp.tile([64, 2 * QT], F32, tag="rc", bufs=4)
                    for i, h in enumerate(heads):
                        nc.vector.tensor_copy(lr[:, QT * i:QT * (i + 1)],
                                              stgs[h][D:D + 1, :])
                    for i, h in enumerate(heads):
                        bc = ps.tile([64, QT], F32, tag="pj", bufs=2,
                                     name=f"bc_{pair}_{j}_{h}")
                        nc.tensor.matmul(bc[:], lhsT=ones[:],
                                         rhs=lr[:, QT * i:QT * (i + 1)],
                                         start=True, stop=True)
                        nc.vector.reciprocal_approx_fast(
                            rc[:, QT * i:QT * (i + 1)], bc[:])
                    for i, h in enumerate(heads):
                        g2, po = h // 2, 64 * (h % 2)
                        nc.vector.tensor_mul(
                            ctxt_sb[po:po + 64, g2, QT * j:QT * (j + 1)],
                            stgs[h][0:D, :], rc[:, QT * i:QT * (i + 1)])

                if last:
                    norm()
                    return None
                return norm

            def wo_ec(tt, ec):
                # one e-chunk of the partial out^T[e, tt-slice] = Wo_s.T@ctx^T
                pp = ps.tile([128, QT], F32, tag="pj", bufs=2,
                             name=f"pp_wo_{ec}_{tt}")
                for g2 in range(2):
                    nc.tensor.matmul(
                        pp[:],
                        lhsT=wo_sb[:, g2, 128 * ec:128 * (ec + 1)],
                        rhs=ctxt_sb[:, g2, QT * tt:QT * (tt + 1)],
                        start=(g2 == 0), stop=(g2 == 1),
                    )
                ot = wp.tile([128, QT], F16, tag="ot", bufs=4)
                if (ec + tt) % 2 == 1:
                    nc.scalar.activation(
                        ot[:], pp[:], mybir.ActivationFunctionType.Copy)
                else:
                    nc.vector.tensor_copy(ot[:], pp[:])
                nc.sync.dma_start(
                    out[128 * ec:128 * (ec + 1), QT * tt:QT * (tt + 1)],
                    ot[:])

            def WO(tt):
                return [(lambda ec=ec: wo_ec(tt, ec)) for ec in range(EC)]

            # pair-0: projections paced per token block so attention starts
            # as soon as block 0 lands instead of after the full xT stream
            def V(t):
                return lambda: proj_v(t)

            def KQ(w, o, g2, j):
                return lambda: proj_kq(w, o, g2, j)

            proj_kq(wk_sb, kt_sb, 0, 0)
            proj_kq(wq_sb, qt_sb, 0, 0)
            pn = attention(0, 0, injects=[V(0), V(1), V(2), V(3),
                                          KQ(wk_sb, kt_sb, 0, 1),
                                          KQ(wq_sb, qt_sb, 0, 1)])
            pn = attention(0, 1, injects=[V(4), V(5), V(6), V(7),
                                          KQ(wk_sb, kt_sb, 0, 2),
                                          KQ(wq_sb, qt_sb, 0, 2)],
                           prev_norm=pn)
            pn = attention(0, 2, injects=[V(8), V(9), V(10), V(11),
                                          KQ(wk_sb, kt_sb, 0, 3),
                                          KQ(wq_sb, qt_sb, 0, 3)],
                           prev_norm=pn)
            pn = attention(0, 3, injects=[V(12), V(13), V(14), V(15),
                                          KQ(wk_sb, kt_sb, 1, 0),
                                          KQ(wk_sb, kt_sb, 1, 1),
                                          KQ(wq_sb, qt_sb, 1, 0)],
                           prev_norm=pn)
            # pair-1: wo units injected one call later than their tt is
            # normalized, so the tail is just attention(1,3)'s norm + wo(3).
            # K-projections for the later pair-1 tiles ride in the small
            # early calls instead of overloading attention(0,3)'s window.
            pn = attention(1, 0, injects=[KQ(wk_sb, kt_sb, 1, 2)],
                           prev_norm=pn)
            proj_kq(wq_sb, qt_sb, 1, 1)
            pn = attention(1, 1, injects=[KQ(wk_sb, kt_sb, 1, 3)],
                           prev_norm=pn)
            proj_kq(wq_sb, qt_sb, 1, 2)
            pn = attention(1, 2, injects=WO(0), prev_norm=pn)
            proj_kq(wq_sb, qt_sb, 1, 3)
            attention(1, 3, injects=WO(1) + WO(2), prev_norm=pn, last=True)
            for ec in range(EC):
                wo_ec(3, ec)
    nc.compile()
    return nc


def _causal_mask():
    # the [128,128] lower-tri block mask shared by all diagonal chunks
    k = np.arange(128)[:, None]
    q = np.arange(128)[None, :]
    return (k <= q).astype(np.float16)


def _w_in(w):
    # [E, FS] -> [128 p, EC chunks, FS] (e = c*128 + p)
    return np.ascontiguousarray(
        w.reshape(EC, 128, FS).transpose(1, 0, 2)).astype(np.float16)


def _wo_in(w):
    # [FS, E] -> [128 p, 2 g, E] (f = g*128 + p)
    return np.ascontiguousarray(
        w.reshape(2, 128, E).transpose(1, 0, 2)).astype(np.float16)


def make_in_maps(x, Wq, Wk, Wv, Wo):
    msk = _causal_mask()
    in_maps = []
    for b in range(B):
        xT_f = np.ascontiguousarray(np.asarray(x[b]).T).astype(np.float16)
        xT_h = np.ascontiguousarray(                     # [tb, c, 128, 512]
            xT_f.reshape(EC, 128, NQ, QT).transpose(2, 0, 1, 3))
        for g in range(4):
            s = slice(g * FS, (g + 1) * FS)
            in_maps.append({
                "xT": xT_h,
                "wq": _w_in(Wq[:, s]),
                "wk": _w_in(Wk[:, s]),
                "wv": _w_in(Wv[:, s]),
                "wo": _wo_in(np.ascontiguousarray(Wo[s, :])),
                "msk": msk,
            })
    return in_maps


def kernel(x, Wq, Wk, Wv, Wo, bo):
    global _CACHED_NC
    x = np.asarray(x, np.float32)
    Wq = np.asarray(Wq, np.float32)
    Wk = np.asarray(Wk, np.float32)
    Wv = np.asarray(Wv, np.float32)
    Wo = np.asarray(Wo, np.float32)
    bo = np.asarray(bo, np.float32)

    if _CACHED_NC is None:
        _CACHED_NC = build()
    nc = _CACHED_NC

    in_maps = make_in_maps(x, Wq, Wk, Wv, Wo)
    res = run_bass_kernel_spmd(nc, in_maps, core_ids=list(range(8)))

    out = np.empty((B, C, E), np.float32)
    for b in range(B):
        acc = res.results[b * 4 + 0]["out"].astype(np.float32)
        for g in range(1, 4):
            acc += res.results[b * 4 + g]["out"]
        out[b] = acc.T + bo          # kernel emits out^T
    return out



# revision 58
# speedup vs baseline: 1.0856x; 1.0856x over previous
"""Multi-head causal attention (B=2, C=2048, E=1024, H=16) on 8 NeuronCores.

Sharding: tensor-parallel over (batch, head-group): core = b*4 + g handles
batch b and heads [4g, 4g+4). Each core computes Q^T/K^T/V projections for
its 4 heads, causal attention, and its partial output projection
ctx_slice @ Wo_slice -> [2048, 1024]. Host sums the 4 partials per batch
(the tensor-parallel all-reduce, done at unshard time) and adds bo.

Dataflow is fully transposed so no on-device transposes are needed:
  Q^T = Wq_s.T @ x^T          [256 f, 2048 t]   (f = head-local features)
  K^T = Wk_s.T @ x^T          [256 f, 2048 t]
  V   = x    @ Wv_s           [2048 t, 256 f]  (natural layout, + ones col)
  s^T = K^T_h.T @ Q^T_h       [128 k, 512 q] per (head, k-chunk, q-tile)
  P^T = exp(s^T / 32) * mask  (no max-subtraction: |scores/32| < ~2.5)
  ctx_aug^T = V_aug.T @ P^T   [65, 512], row 64 = softmax normalizer l
  ctx^T = ctx_aug^T[0:64] * (1/l)  (broadcast via gpsimd)
  out_partial = ctx^T.T @ Wo_s     [2048, 1024] fp32

All matmul operands are fp16 (PE upconverts to FP22, accumulates fp32 in
PSUM): end-to-end max rel err vs fp64 reference is ~4e-4.
"""
import numpy as np

import concourse.bass as bass
import concourse.tile as tile
from concourse import bacc, mybir
from concourse.bass_utils import run_bass_kernel_spmd

F16 = mybir.dt.float16
F32 = mybir.dt.float32

B, C, E, H = 2, 2048, 1024, 16
NH = 4              # heads per core
D = 64              # head dim
FS = NH * D         # 256 features per core
EC = E // 128       # 8 e-chunks
QT = 512            # q tile size
NQ = C // QT        # 4 q tiles
KC = C // 128       # 16 k chunks
TC = C // 128       # 16 token chunks
SCALE = 1.0 / np.sqrt(np.float32(E))  # note: module scales by sqrt(E)

_CACHED_NC = None


def build():
    nc = bacc.Bacc("TRN2", target_bir_lowering=False, debug=False, num_devices=8)
    # xT arrives token-block-major: [tb, c, 128, 512] so each (tb, c) piece
    # is one contiguous 128KB DMA and projections can pace to block arrival
    xT = nc.dram_tensor("xT", [NQ, EC, 128, QT], F16, kind="ExternalInput")
    # weights/masks arrive pre-laid-out for contiguous DMA:
    # wq/wk/wv: [128, EC, FS]; wo: [128, 2, E]; msk: [128, 4, QT]
    wq = nc.dram_tensor("wq", [128, EC, FS], F16, kind="ExternalInput")
    wk = nc.dram_tensor("wk", [128, EC, FS], F16, kind="ExternalInput")
    wv = nc.dram_tensor("wv", [128, EC, FS], F16, kind="ExternalInput")
    wo = nc.dram_tensor("wo", [128, 2, E], F16, kind="ExternalInput")
    msk = nc.dram_tensor("msk", [128, 128], F16, kind="ExternalInput")
    out = nc.dram_tensor("out", [E, C], F16, kind="ExternalOutput")  # out^T

    with tile.TileContext(nc) as tc:
        with tc.tile_pool(name="const", bufs=1) as cp, \
             tc.tile_pool(name="work", bufs=1) as wp, \
             tc.tile_pool(name="ps", bufs=1, space="PSUM") as ps:
            # ---- resident SBUF tensors ----
            xT_sb = cp.tile([128, EC, C], F16)
            wq_sb = cp.tile([128, EC, FS], F16)
            wk_sb = cp.tile([128, EC, FS], F16)
            wv_sb = cp.tile([128, EC, FS], F16)
            wo_sb = cp.tile([128, 2, E], F16)
            msk_sb = cp.tile([128, 128], F16)
            qt_sb = cp.tile([128, 2, C], F16)
            kt_sb = cp.tile([128, 2, C], F16)
            v_sb = cp.tile([128, TC, NH * (D + 1)], F16)   # +1: ones col per head
            ctxt_sb = cp.tile([128, 2, C], F16)

            # ---- input DMAs in first-consumption order.  DMA data is
            # striped across all engines, so arrival ~ emission order at
            # the aggregate rate: wk ~9.4us, tb0 ~12.2, wq ~13.6, ...
            nc.sync.dma_start(wk_sb[:], wk[:])
            for c in range(EC):
                nc.sync.dma_start(xT_sb[:, c, 0:QT], xT[0, c])
            nc.sync.dma_start(wq_sb[:], wq[:])
            nc.sync.dma_start(msk_sb[:], msk[:])
            nc.sync.dma_start(wv_sb[:], wv[:])
            for tb in range(1, NQ):
                for c in range(EC):
                    nc.sync.dma_start(
                        xT_sb[:, c, QT * tb:QT * (tb + 1)], xT[tb, c])
            nc.sync.dma_start(wo_sb[:], wo[:])
            nc.vector.memset(v_sb[:], 1.0)  # ones cols survive the V copy

            # ---- PE warm-up: ~7us of dummy matmuls during the input-DMA
            # window so HAM un-throttles before the real stream starts
            wu = wp.tile([128, 512], F16, tag="wu", bufs=1)
            nc.vector.memset(wu[:], 0.5)
            ones = wp.tile([1, 64], F16, tag="ones", bufs=1)
            nc.vector.memset(ones[:], 1.0)
            wups = ps.tile([128, 512], F32, tag="ctx", bufs=2, name="wups")
            for i in range(16):
                nc.tensor.matmul(wups[:], lhsT=wu[:, 0:128], rhs=wu[:],
                                 start=True, stop=True)

            # ---- projection building blocks (emitted interleaved below) ----
            def proj_kq(w_sb, o_sb, g2, j):
                pp = ps.tile([128, QT], F32, tag="pj", bufs=2,
                             name=f"pp_{o_sb.name}_{g2}_{j}")
                for c in range(EC):
                    nc.tensor.matmul(
                        pp[:],
                        lhsT=w_sb[:, c, 128 * g2:128 * (g2 + 1)],
                        rhs=xT_sb[:, c, QT * j:QT * (j + 1)],
                        start=(c == 0), stop=(c == EC - 1),
                    )
                nc.vector.tensor_copy(o_sb[:, g2, QT * j:QT * (j + 1)], pp[:])

            def proj_v(t):
                pp = ps.tile([128, FS], F32, tag="pj", bufs=2,
                             name=f"pp_v_{t}")
                for c in range(EC):
                    nc.tensor.matmul(
                        pp[:],
                        lhsT=xT_sb[:, c, 128 * t:128 * (t + 1)],
                        rhs=wv_sb[:, c, :],
                        start=(c == 0), stop=(c == EC - 1),
                    )
                nc.vector.tensor_copy(
                    v_sb[:, t, :].rearrange("p (h x) -> p h x", h=NH)[:, :, 0:D],
                    pp[:].rearrange("p (h d) -> p h d", h=NH),
                )

            # ---- attention: head pairs (0,1)/(2,3); both heads' score tiles
            # ---- share one [128, 2*QT] psum so exp+mask are single wide ops
            def emit_scores(heads, j, c):
                """s^T pair -> one exp -> (one mask) -> fp16 P^T [128, 2*QT].

                Diagonal chunks (c = 4j + r): queries q < 128r are fully
                masked, so all work is restricted to q in [128r, QT)."""
                q0 = 128 * (c - 4 * j) if c >= 4 * j else 0
                qw = QT - q0
                st = ps.tile([128, 2 * QT], F32, tag="big", bufs=2,
                             name=f"st_{heads[0]}_{j}_{c}")
                for i, h in enumerate(heads):
                    g2, po = h // 2, 64 * (h % 2)
                    nc.tensor.matmul(
                        st[:, QT * i + q0:QT * (i + 1)],
                        lhsT=kt_sb[po:po + 64, g2, 128 * c:128 * (c + 1)],
                        rhs=qt_sb[po:po + 64, g2, QT * j + q0:QT * (j + 1)],
                        start=True, stop=True,
                    )
                pt = wp.tile([128, 2 * QT], F16, tag="pt", bufs=6)
                st3 = st[:].rearrange("p (b q) -> p b q", b=2)[:, :, q0:QT]
                pt3 = pt[:].rearrange("p (b q) -> p b q", b=2)[:, :, q0:QT]
                nc.scalar.activation(
                    pt3, st3, mybir.ActivationFunctionType.Exp, scale=SCALE)
                if c >= 4 * j:  # mask the diagonal 128-wide block in place
                    # (the causal mask within that block is the same
                    # [128,128] lower-tri for every diagonal chunk)
                    ptd = pt[:].rearrange("p (b q) -> p b q", b=2)[
                        :, :, q0:q0 + 128]
                    nc.vector.tensor_mul(
                        ptd, ptd,
                        msk_sb[:].unsqueeze(1).broadcast_to([128, 2, 128]))
                return pt

            def attention(pair, j, injects=(), last=False):
                """injects: callables emitted one per k-chunk iteration --
                extra tensor work scheduled into exp-bound PE bubbles.
                last: skip the psum->sbuf staging (nothing queues behind the
                final call, so shorten the normalize chain instead)."""
                heads = (2 * pair, 2 * pair + 1)
                nk = 4 * (j + 1)   # causal: k chunks 0..nk-1
                ctx_ps = {h: ps.tile([128, QT], F32, tag="ctx", bufs=2,
                                     name=f"ctx_{pair}_{j}_{h}")
                          for h in heads}
                pts = {}
                depth = min(2, nk)
                for c in range(depth):          # software-pipeline prologue
                    pts[c] = emit_scores(heads, j, c)
                inj = list(injects)
                for c in range(nk):
                    if c < len(inj):
                        inj[c]()
                    if c + depth < nk:
                        pts[c + depth] = emit_scores(heads, j, c + depth)
                    pt = pts.pop(c)
                    q0 = 128 * (c - 4 * j) if c >= 4 * j else 0
                    for i, h in enumerate(heads):
                        nc.tensor.matmul(
                            ctx_ps[h][0:D + 1, q0:QT],
                            lhsT=v_sb[:, c, (D + 1) * h:(D + 1) * (h + 1)],
                            rhs=pt[:, QT * i + q0:QT * (i + 1)],
                            start=(c == 0), stop=(c == nk - 1),
                        )
                for fn in inj[nk:]:
                    fn()
                # normalize: ctx^T[d, q] * (1/l[q]).  Stage both heads' psum
                # to SBUF first (frees the ctx psum slots fast so following
                # matmuls aren't gated on the slow recip chain).  l is then
                # broadcast across partitions with a rank-1 PE matmul into
                # the just-freed ctx psum slots -- lower latency than the
                # gpsimd partition_broadcast.
                if last:
                    stgs = {h: ctx_ps[h] for h in heads}
                else:
                    stgs = {}
                    for h in heads:
                        stg = wp.tile([D + 1, QT], F32, tag="stg", bufs=4,
                                      name=f"stg_{pair}_{j}_{h}")
                        nc.vector.tensor_copy(stg[:], ctx_ps[h][0:D + 1, :])
                        stgs[h] = stg
                lr = wp.tile([1, 2 * QT], F16, tag="lr", bufs=4)
                rc = wp.tile([64, 2 * QT], F32, tag="rc", bufs=4)
                for i, h in enumerate(heads):
                    nc.vector.tensor_copy(lr[:, QT * i:QT * (i + 1)],
                                          stgs[h][D:D + 1, :])
                for i, h in enumerate(heads):
                    # when last=True the muls below still hold the ctx slots,
                    # so bc must come from a different tag to avoid a cycle
                    bc = ps.tile([64, QT], F32, tag="pj" if last else "ctx",
                                 bufs=2, name=f"bc_{pair}_{j}_{h}")
                    nc.tensor.matmul(bc[:], lhsT=ones[:],
                                     rhs=lr[:, QT * i:QT * (i + 1)],
                                     start=True, stop=True)
                    nc.vector.reciprocal_approx_fast(
                        rc[:, QT * i:QT * (i + 1)], bc[:])
                for i, h in enumerate(heads):
                    g2, po = h // 2, 64 * (h % 2)
                    nc.vector.tensor_mul(
                        ctxt_sb[po:po + 64, g2, QT * j:QT * (j + 1)],
                        stgs[h][0:D, :], rc[:, QT * i:QT * (i + 1)])

            def wo_ec(tt, ec):
                # one e-chunk of the partial out^T[e, tt-slice] = Wo_s.T@ctx^T
                pp = ps.tile([128, QT], F32, tag="pj", bufs=2,
                             name=f"pp_wo_{ec}_{tt}")
                for g2 in range(2):
                    nc.tensor.matmul(
                        pp[:],
                        lhsT=wo_sb[:, g2, 128 * ec:128 * (ec + 1)],
                        rhs=ctxt_sb[:, g2, QT * tt:QT * (tt + 1)],
                        start=(g2 == 0), stop=(g2 == 1),
                    )
                ot = wp.tile([128, QT], F16, tag="ot", bufs=4)
                if (ec + tt) % 2 == 1:
                    nc.scalar.activation(
                        ot[:], pp[:], mybir.ActivationFunctionType.Copy)
                else:
                    nc.vector.tensor_copy(ot[:], pp[:])
                nc.sync.dma_start(
                    out[128 * ec:128 * (ec + 1), QT * tt:QT * (tt + 1)],
                    ot[:])

            def WO(tt):
                return [(lambda ec=ec: wo_ec(tt, ec)) for ec in range(EC)]

            # pair-0: projections paced per token block so attention starts
            # as soon as block 0 lands instead of after the full xT stream
            def V(t):
                return lambda: proj_v(t)

            def KQ(w, o, g2, j):
                return lambda: proj_kq(w, o, g2, j)

            proj_kq(wk_sb, kt_sb, 0, 0)
            proj_kq(wq_sb, qt_sb, 0, 0)
            attention(0, 0, injects=[V(0), V(1), V(2), V(3),
                                     KQ(wk_sb, kt_sb, 0, 1),
                                     KQ(wq_sb, qt_sb, 0, 1)])
            attention(0, 1, injects=[V(4), V(5), V(6), V(7),
                                     KQ(wk_sb, kt_sb, 0, 2),
                                     KQ(wq_sb, qt_sb, 0, 2)])
            attention(0, 2, injects=[V(8), V(9), V(10), V(11),
                                     KQ(wk_sb, kt_sb, 0, 3),
                                     KQ(wq_sb, qt_sb, 0, 3)])
            attention(0, 3, injects=[V(12), V(13), V(14), V(15),
                                     KQ(wk_sb, kt_sb, 1, 0),
                                     KQ(wk_sb, kt_sb, 1, 1),
                                     KQ(wq_sb, qt_sb, 1, 0)])
            # pair-1: wo units injected one call later than their tt is
            # normalized, so the tail is just attention(1,3)'s norm + wo(3).
            # K-projections for the later pair-1 tiles ride in the small
            # early calls instead of overloading attention(0,3)'s window.
            attention(1, 0, injects=[KQ(wk_sb, kt_sb, 1, 2)])
            proj_kq(wq_sb, qt_sb, 1, 1)
            attention(1, 1, injects=[KQ(wk_sb, kt_sb, 1, 3)])
            proj_kq(wq_sb, qt_sb, 1, 2)
            attention(1, 2, injects=WO(0))
            proj_kq(wq_sb, qt_sb, 1, 3)
            attention(1, 3, injects=WO(1) + WO(2), last=True)
            for ec in range(EC):
                wo_ec(3, ec)
    nc.compile()
    return nc


def _causal_mask():
    # the [128,128] lower-tri block mask shared by all diagonal chunks
    k = np.arange(128)[:, None]
    q = np.arange(128)[None, :]
    return (k <= q).astype(np.float16)


def _w_in(w):
    # [E, FS] -> [128 p, EC chunks, FS] (e = c*128 + p)
    return np.ascontiguousarray(
        w.reshape(EC, 128, FS).transpose(1, 0, 2)).astype(np.float16)


def _wo_in(w):
    # [FS, E] -> [128 p, 2 g, E] (f = g*128 + p)
    return np.ascontiguousarray(
        w.reshape(2, 128, E).transpose(1, 0, 2)).astype(np.float16)


def make_in_maps(x, Wq, Wk, Wv, Wo):
    msk = _causal_mask()
    in_maps = []
    for b in range(B):
        xT_f = np.ascontiguousarray(np.asarray(x[b]).T).astype(np.float16)
        xT_h = np.ascontiguousarray(                     # [tb, c, 128, 512]
            xT_f.reshape(EC, 128, NQ, QT).transpose(2, 0, 1, 3))
        for g in range(4):
            s = slice(g * FS, (g + 1) * FS)
            in_maps.append({
                "xT": xT_h,
                "wq": _w_in(Wq[:, s]),
                "wk": _w_in(Wk[:, s]),
                "wv": _w_in(Wv[:, s]),
                "wo": _wo_in(np.ascontiguousarray(Wo[s, :])),
                "msk": msk,
            })
    return in_maps


def kernel(x, Wq, Wk, Wv, Wo, bo):
    global _CACHED_NC
    x = np.asarray(x, np.float32)
    Wq = np.asarray(Wq, np.float32)
    Wk = np.asarray(Wk, np.float32)
    Wv = np.asarray(Wv, np.float32)
    Wo = np.asarray(Wo, np.float32)
    bo = np.asarray(bo, np.float32)

    if _CACHED_NC is None:
        _CACHED_NC = build()
    nc = _CACHED_NC

    in_maps = make_in_maps(x, Wq, Wk, Wv, Wo)
    res = run_bass_kernel_spmd(nc, in_maps, core_ids=list(range(8)))

    out = np.empty((B, C, E), np.float32)
    for b in range(B):
        acc = res.results[b * 4 + 0]["out"].astype(np.float32)
        for g in range(1, 4):
            acc += res.results[b * 4 + g]["out"]
        out[b] = acc.T + bo          # kernel emits out^T
    return out

